# revision 8
# baseline (speedup 1.0000x reference)
"""MoE expert-pool kernel for 8 TRN2 NeuronCores (hidden-dim sharded).

Strategy (v2, F-sharded):
  - Instead of one expert per core (which pads every core to the max
    expert token count), each core holds an F/8 = 512-wide slice of
    ALL 8 experts' weights and processes ALL routed (token, expert)
    rows, producing a partial y contracted over its F-slice. The host
    sums the 8 bf16 partials. Per-core PE work is exactly
    total_rows/8 column-equivalents regardless of routing skew.
  - Host-side routing dedups (token, expert) pairs: a token that picks
    the same expert in multiple top-k slots becomes ONE row with an
    integer multiplicity applied at the host scatter. Rows are sorted
    by expert into contiguous segments; segment sizes are baked into
    the NEFF (compiled per routing instance, cached by segment tuple).
  - On-device layout is transposed (feature axis on partitions, tokens
    on the free axis): hT = gelu(w1s^T x + b1s), yT_partial = w2s^T hT.
    b2 is added on the host (one add per row during the scatter).
  - x is streamed per 512-column block through a rotating SBUF pool
    (the full routed xT no longer fits alongside both weight shards);
    weight slices are DMA'd per expert just ahead of their segment.
"""

import numpy as np

_REPO = "/opt/trn_rl_repo"

_D = 1024  # d_model
_F = 4096  # d_ff
_P = 128   # partitions
_E = 8     # experts
_KD = _D // _P        # 8 contraction tiles along D
_FS = _F // 8         # 512: per-core F-slice width
_FT = _FS // _P       # 4 f-tiles per core
_NB = 512             # token block = one fp32 PSUM bank
_W1COLS = _E * _FT * _KD * _P   # w1 shard SBUF cols  [e][f][k][128]
_W2COLS = _E * _KD * _FT * _P   # w2 shard SBUF cols  [e][d][ft][128]

_NCORES = 8

_cache = {}
LAST_RESULT = None


def _ensure_path():
    import sys
    if _REPO not in sys.path:
        sys.path.insert(0, _REPO)


def _ensure_axon_hooks():
    """The container's `antenv` stub lacks `axon_hooks`, which
    bass_utils imports unconditionally on the traced (BASS_TRACE) axon
    path. Provide the missing get/set registry and register the NTFF
    ctypes hook the boot shim would have installed."""
    try:
        import antenv.axon_hooks  # noqa: F401
        return
    except ImportError:
        pass
    import sys
    import types
    mod = types.ModuleType("antenv.axon_hooks")
    mod._hook = None

    def set_axon_ntff_profile_hook(h):
        mod._hook = h

    def get_axon_ntff_profile_hook():
        return mod._hook

    mod.set_axon_ntff_profile_hook = set_axon_ntff_profile_hook
    mod.get_axon_ntff_profile_hook = get_axon_ntff_profile_hook
    sys.modules["antenv.axon_hooks"] = mod
    try:
        import antenv
        antenv.axon_hooks = mod
    except ImportError:
        pass
    try:
        from trn_agent_boot.trn_boot import _ntff_profile_via_ctypes
        hook = _ntff_profile_via_ctypes("/opt/axon/libaxon_pjrt.so")
        if hook is not None:
            mod._hook = hook
    except Exception:
        pass


_FIRSTNB = 160  # small leading block: PE starts after ~0.6 MB of DMA


def _plan_blocks(segs):
    """Split each expert segment into <=512-col blocks (one fp32 PSUM
    bank), sizes equal-ish multiples of 8. The very first block is
    carved small so matmul1 starts early in the DMA ramp.
    Returns [(e, col0, nb)]."""
    blocks = []
    col = 0
    first = True
    for e, se in enumerate(segs):
        if se == 0:
            continue
        sizes = []
        if first and se > 2 * _FIRSTNB:
            sizes.append(_FIRSTNB)
            se -= _FIRSTNB
        first = False
        nblk = -(-se // _NB)
        base = se // nblk // 8 * 8
        sizes += [base] * nblk
        extra = se - base * nblk
        i = 0
        while extra > 0:
            step = min(8, extra)
            sizes[len(sizes) - nblk + i % nblk] += step
            extra -= step
            i += 1
        for nb in sizes:
            blocks.append((e, col, nb))
            col += nb
    return blocks, col


def _build(segs):
    _ensure_path()
    from concourse import bacc, mybir
    from concourse.tile import TileContext

    dt = mybir.dt
    AF = mybir.ActivationFunctionType

    blocks, Ctot = _plan_blocks(segs)
    assert blocks, "no routed tokens"

    # Bacc (not plain Bass): its compile() pass splits multi-sem waits
    # into event-semaphore instructions (TRN2 allows 1 wait/instruction).
    nc = bacc.Bacc("TRN2", target_bir_lowering=False, debug=False)
    xT = nc.declare_dram_parameter("xT", [_D, Ctot], dt.bfloat16, isOutput=False)
    w1 = nc.declare_dram_parameter("w1", [_P, _W1COLS], dt.bfloat16,
                                   isOutput=False)
    w2 = nc.declare_dram_parameter("w2", [_P, _W2COLS], dt.bfloat16,
                                   isOutput=False)
    bia = nc.declare_dram_parameter("bias", [_P, _E * _FT], dt.float32,
                                    isOutput=False)
    yT = nc.declare_dram_parameter("yT", [_D, Ctot], dt.bfloat16, isOutput=True)

    xTv = xT.rearrange("(k p) c -> p k c", p=_P)
    yTv = yT.rearrange("(d p) c -> p d c", p=_P)

    with TileContext(nc) as tc:
        with (
            tc.tile_pool(name="persist", bufs=1) as pers,
            tc.tile_pool(name="xpool", bufs=4) as xp,
            tc.tile_pool(name="hpool", bufs=4) as hp,
            tc.tile_pool(name="ypool", bufs=3) as yp,
            tc.tile_pool(name="ph", bufs=4, space="PSUM") as php,
            tc.tile_pool(name="py", bufs=4, space="PSUM") as pyp,
        ):
            w1s = pers.tile([_P, _W1COLS], dt.bfloat16, name="w1s")
            w2s = pers.tile([_P, _W2COLS], dt.bfloat16, name="w2s")
            bs = pers.tile([_P, _E * _FT], dt.float32, name="bs")

            # HAM warm-up: the PE clock sits at 1.2 GHz until ~3.4 us
            # of sustained activity. Fill the otherwise-idle DMA ramp
            # (~8.5 us before the first real matmul) with dummy matmuls
            # on a memset tile so real work starts at 2.4 GHz.
            warm = pers.tile([_P, _P], dt.bfloat16, name="warm")
            nc.vector.memset(warm[:, :], 0.0)
            wp = php.tile([_P, _NB], dt.float32, name="ph", tag="ph")
            for _ in range(32):
                nc.tensor.matmul(wp[:, :_P], lhsT=warm[:, :],
                                 rhs=warm[:, :], start=True, stop=True)

            xs_tiles = [xp.tile([_P, _KD * _NB], dt.bfloat16,
                                name="xs", tag="xs") for _ in blocks]

            def dma_xs(bi):
                e, c0, nb = blocks[bi]
                nc.sync.dma_start(
                    out=xs_tiles[bi][:, :_KD * nb].rearrange(
                        "p (k c) -> p k c", k=_KD),
                    in_=xTv[:, :, c0:c0 + nb])

            def dma_w1(e, nchunks):
                off, span = e * _FT * _KD * _P, _FT * _KD * _P
                cw = span // nchunks
                for i in range(nchunks):
                    nc.sync.dma_start(
                        out=w1s[:, off + i * cw: off + (i + 1) * cw],
                        in_=w1[:, off + i * cw: off + (i + 1) * cw])

            def dma_w2(e, nchunks):
                off, span = e * _KD * _FT * _P, _KD * _FT * _P
                cw = span // nchunks
                for i in range(nchunks):
                    nc.sync.dma_start(
                        out=w2s[:, off + i * cw: off + (i + 1) * cw],
                        in_=w2[:, off + i * cw: off + (i + 1) * cw])

            # DMA triggers execute SERIALLY on the sync engine in
            # program order, and a trigger blocks the stream until its
            # wait clears. So: never front-load the whole input stream
            # — keep only a _LOOKAHEAD-block input window ahead of
            # compute, and issue each block's output DMA right after
            # its compute so it triggers as soon as its data is ready.
            issued = set()

            def issue_inputs(bi):
                e = blocks[bi][0]
                if e not in issued:
                    dma_w1(e, nchunks=1)
                    dma_w2(e, nchunks=1)
                    issued.add(e)
                dma_xs(bi)

            _LOOKAHEAD = 4
            e0 = blocks[0][0]
            dma_xs(0)
            dma_w1(e0, nchunks=_FT)  # f-chunked: m1 starts after chunk 0
            nc.sync.dma_start(out=bs[:, :], in_=bia[:, :])
            if len(blocks) > 1:
                issue_inputs(1)  # x is work-dense: ahead of w2 at ramp
            dma_w2(e0, nchunks=2)
            issued.add(e0)
            for bi in range(2, min(_LOOKAHEAD, len(blocks))):
                issue_inputs(bi)

            # Ramp burst: run matmul1 of the first two blocks before the
            # first matmul2 — converts early DMA bytes into PE work
            # while w2[e0] is still in flight.
            _BURST = min(2, len(blocks))
            sched = [(b, "m1") for b in range(_BURST)]
            sched += [(b, "m2") for b in range(_BURST)]
            for b in range(_BURST, len(blocks)):
                sched += [(b, "m1"), (b, "m2")]

            hts_tiles = {}
            for bi, phase in sched:
                e, c0, nb = blocks[bi]
                if phase == "m1":
                    if bi + _LOOKAHEAD < len(blocks):
                        issue_inputs(bi + _LOOKAHEAD)
                    xs = xs_tiles[bi]
                    hts = hp.tile([_P, _FT * _NB], dt.bfloat16,
                                  name="hts", tag="hts")
                    hts_tiles[bi] = hts
                    for f in range(_FT):
                        ph = php.tile([_P, _NB], dt.float32,
                                      name="ph", tag="ph")
                        woff = (e * _FT + f) * _KD * _P
                        for k in range(_KD):
                            nc.tensor.matmul(
                                ph[:, :nb],
                                lhsT=w1s[:, woff + k * _P:
                                         woff + (k + 1) * _P],
                                rhs=xs[:, k * nb:(k + 1) * nb],
                                start=(k == 0), stop=(k == _KD - 1))
                        nc.scalar.activation(
                            hts[:, f * _NB: f * _NB + nb], ph[:, :nb],
                            AF.Gelu, bias=bs[:, e * _FT + f: e * _FT + f + 1])
                    continue
                hts = hts_tiles.pop(bi)
                last = bi == len(blocks) - 1
                yt = yp.tile([_P, _KD * _NB], dt.bfloat16, name="yt", tag="yt")
                for d in range(_KD):
                    # The very last output gates the kernel tail; split
                    # it column-wise so half drains earlier.
                    if last and d == _KD - 1 and nb >= 64:
                        h1 = (nb // 2 + 7) // 8 * 8
                        spans = [(0, h1), (h1, nb - h1)]
                    else:
                        spans = [(0, nb)]
                    for (s0, cw) in spans:
                        py = pyp.tile([_P, _NB], dt.float32,
                                      name="py", tag="py")
                        voff = (e * _KD + d) * _FT * _P
                        for ft in range(_FT):
                            nc.tensor.matmul(
                                py[:, :cw],
                                lhsT=w2s[:, voff + ft * _P:
                                         voff + (ft + 1) * _P],
                                rhs=hts[:, ft * _NB + s0: ft * _NB + s0 + cw],
                                start=(ft == 0), stop=(ft == _FT - 1))
                        nc.vector.tensor_scalar_add(
                            yt[:, d * nb + s0: d * nb + s0 + cw],
                            py[:, :cw], 0.0)
                        if last:
                            nc.sync.dma_start(
                                out=yT[d * _P:(d + 1) * _P,
                                       c0 + s0:c0 + s0 + cw],
                                in_=yt[:, d * nb + s0: d * nb + s0 + cw])
                if not last:
                    nc.sync.dma_start(
                        out=yTv[:, :, c0:c0 + nb],
                        in_=yt[:, :_KD * nb].rearrange(
                            "p (d c) -> p d c", d=_KD))
    nc.finalize()
    return nc


def kernel(x, expert_indices, w1, b1, w2, b2):
    global LAST_RESULT
    _ensure_path()
    _ensure_axon_hooks()
    import ml_dtypes
    from concourse.bass_utils import run_bass_kernel_spmd

    bf16 = ml_dtypes.bfloat16
    x = np.asarray(x)
    idxs = np.asarray(expert_indices)
    w1 = np.asarray(w1, dtype=np.float32)
    b1 = np.asarray(b1, dtype=np.float32)
    w2 = np.asarray(w2, dtype=np.float32)
    b2 = np.asarray(b2, dtype=np.float32)

    B, S, D = x.shape
    T = B * S
    E = w1.shape[0]
    K = idxs.shape[-1]
    assert D == _D and w1.shape[2] == _F and E == _E

    xf = np.ascontiguousarray(x.reshape(T, D).astype(np.float32))
    idx = idxs.reshape(T, K)

    # Deduplicated routing: one row per (token, expert) with integer
    # multiplicity (a token picking the same expert in several top-k
    # slots is computed once and scaled at the scatter).
    toks, wts, offs, cnts = [], [], [], []
    off = 0
    for e in range(E):
        m = (idx == e).sum(axis=1)
        te = np.nonzero(m)[0]
        toks.append(te)
        wts.append(m[te].astype(np.float32))
        cnts.append(len(te))
        offs.append(off)
        off += (len(te) + 7) // 8 * 8
    segs = tuple((c + 7) // 8 * 8 for c in cnts)
    Ctot = sum(segs)

    xTfull = np.zeros((_D, Ctot), dtype=bf16)
    for e in range(E):
        if cnts[e]:
            xTfull[:, offs[e]:offs[e] + cnts[e]] = \
                xf[toks[e]].T.astype(bf16)

    # Per-core weight shards: core c takes F columns [c*512, (c+1)*512)
    # of every expert, pre-arranged into the exact SBUF layouts:
    #   w1s: [p, e, f(4), k(8), 128]   (lhsT tiles for matmul1)
    #   w2s: [p, e, d(8), ft(4), 128]  (lhsT tiles for matmul2)
    #   bs : [p, e, f(4)]              (b1 per-partition scalars)
    in_maps = []
    for c in range(_NCORES):
        fs = slice(c * _FS, (c + 1) * _FS)
        a = w1[:, :, fs].reshape(E, _KD, _P, _FT, _P)
        w1shard = np.ascontiguousarray(
            a.transpose(2, 0, 3, 1, 4).reshape(_P, _W1COLS)).astype(bf16)
        b = w2[:, fs, :].reshape(E, _FT, _P, _KD, _P)
        w2shard = np.ascontiguousarray(
            b.transpose(2, 0, 3, 1, 4).reshape(_P, _W2COLS)).astype(bf16)
        bshard = np.ascontiguousarray(
            b1[:, fs].reshape(E, _FT, _P).transpose(2, 0, 1)
            .reshape(_P, E * _FT)).astype(np.float32)
        in_maps.append({"xT": xTfull, "w1": w1shard, "w2": w2shard,
                        "bias": bshard})

    nc = _cache.get(segs)
    if nc is None:
        nc = _build(segs)
        _cache[segs] = nc

    res = run_bass_kernel_spmd(nc, in_maps, core_ids=list(range(_NCORES)))
    LAST_RESULT = res

    ysum = np.zeros((_D, Ctot), dtype=np.float32)
    for c in range(_NCORES):
        ysum += np.asarray(res.results[c]["yT"]).astype(np.float32)

    out = np.zeros((T, D), dtype=np.float32)
    for e in range(E):
        n = cnts[e]
        if n:
            out[toks[e]] += wts[e][:, None] * (
                ysum[:, offs[e]:offs[e] + n].T + b2[e][None, :])
    return out.reshape(B, S, D)


# revision 10
# speedup vs baseline: 1.0324x; 1.0324x over previous
"""MoE expert-pool kernel for 8 TRN2 NeuronCores (hidden-dim sharded).

Strategy (v2, F-sharded):
  - Instead of one expert per core (which pads every core to the max
    expert token count), each core holds an F/8 = 512-wide slice of
    ALL 8 experts' weights and processes ALL routed (token, expert)
    rows, producing a partial y contracted over its F-slice. The host
    sums the 8 bf16 partials. Per-core PE work is exactly
    total_rows/8 column-equivalents regardless of routing skew.
  - Host-side routing dedups (token, expert) pairs: a token that picks
    the same expert in multiple top-k slots becomes ONE row with an
    integer multiplicity applied at the host scatter. Rows are sorted
    by expert into contiguous segments; segment sizes are baked into
    the NEFF (compiled per routing instance, cached by segment tuple).
  - On-device layout is transposed (feature axis on partitions, tokens
    on the free axis): hT = gelu(w1s^T x + b1s), yT_partial = w2s^T hT.
    b2 is added on the host (one add per row during the scatter).
  - x is streamed per 512-column block through a rotating SBUF pool
    (the full routed xT no longer fits alongside both weight shards);
    weight slices are DMA'd per expert just ahead of their segment.
"""

import numpy as np

_REPO = "/opt/trn_rl_repo"

_D = 1024  # d_model
_F = 4096  # d_ff
_P = 128   # partitions
_E = 8     # experts
_KD = _D // _P        # 8 contraction tiles along D
_FS = _F // 8         # 512: per-core F-slice width
_FT = _FS // _P       # 4 f-tiles per core
_NB = 512             # token block = one fp32 PSUM bank
_W1COLS = _E * _FT * _KD * _P   # w1 shard SBUF cols  [e][f][k][128]
_W2COLS = _E * _KD * _FT * _P   # w2 shard SBUF cols  [e][d][ft][128]

_NCORES = 8

_cache = {}
LAST_RESULT = None


def _ensure_path():
    import sys
    if _REPO not in sys.path:
        sys.path.insert(0, _REPO)


def _ensure_axon_hooks():
    """The container's `antenv` stub lacks `axon_hooks`, which
    bass_utils imports unconditionally on the traced (BASS_TRACE) axon
    path. Provide the missing get/set registry and register the NTFF
    ctypes hook the boot shim would have installed."""
    try:
        import antenv.axon_hooks  # noqa: F401
        return
    except ImportError:
        pass
    import sys
    import types
    mod = types.ModuleType("antenv.axon_hooks")
    mod._hook = None

    def set_axon_ntff_profile_hook(h):
        mod._hook = h

    def get_axon_ntff_profile_hook():
        return mod._hook

    mod.set_axon_ntff_profile_hook = set_axon_ntff_profile_hook
    mod.get_axon_ntff_profile_hook = get_axon_ntff_profile_hook
    sys.modules["antenv.axon_hooks"] = mod
    try:
        import antenv
        antenv.axon_hooks = mod
    except ImportError:
        pass
    try:
        from trn_agent_boot.trn_boot import _ntff_profile_via_ctypes
        hook = _ntff_profile_via_ctypes("/opt/axon/libaxon_pjrt.so")
        if hook is not None:
            mod._hook = hook
    except Exception:
        pass


_FIRSTNB = 160  # small leading block: PE starts after ~0.6 MB of DMA


def _plan_blocks(segs):
    """Split each expert segment into <=512-col blocks (one fp32 PSUM
    bank), sizes equal-ish multiples of 8. The very first block is
    carved small so matmul1 starts early in the DMA ramp.
    Returns [(e, col0, nb)]."""
    blocks = []
    col = 0
    first = True
    for e, se in enumerate(segs):
        if se == 0:
            continue
        sizes = []
        if first and se > 4 * _FIRSTNB:
            sizes += [_FIRSTNB, _FIRSTNB]
            se -= 2 * _FIRSTNB
        first = False
        nblk = -(-se // _NB)
        base = se // nblk // 8 * 8
        sizes += [base] * nblk
        extra = se - base * nblk
        i = 0
        while extra > 0:
            step = min(8, extra)
            sizes[len(sizes) - nblk + i % nblk] += step
            extra -= step
            i += 1
        for nb in sizes:
            blocks.append((e, col, nb))
            col += nb
    return blocks, col


def _build(segs):
    _ensure_path()
    from concourse import bacc, mybir
    from concourse.tile import TileContext

    dt = mybir.dt
    AF = mybir.ActivationFunctionType

    blocks, Ctot = _plan_blocks(segs)
    assert blocks, "no routed tokens"

    # Bacc (not plain Bass): its compile() pass splits multi-sem waits
    # into event-semaphore instructions (TRN2 allows 1 wait/instruction).
    nc = bacc.Bacc("TRN2", target_bir_lowering=False, debug=False)
    xT = nc.declare_dram_parameter("xT", [_D, Ctot], dt.bfloat16, isOutput=False)
    w1 = nc.declare_dram_parameter("w1", [_P, _W1COLS], dt.bfloat16,
                                   isOutput=False)
    w2 = nc.declare_dram_parameter("w2", [_P, _W2COLS], dt.bfloat16,
                                   isOutput=False)
    bia = nc.declare_dram_parameter("bias", [_P, _E * _FT], dt.float32,
                                    isOutput=False)
    yT = nc.declare_dram_parameter("yT", [_D, Ctot], dt.bfloat16, isOutput=True)

    xTv = xT.rearrange("(k p) c -> p k c", p=_P)
    yTv = yT.rearrange("(d p) c -> p d c", p=_P)

    with TileContext(nc) as tc:
        with (
            tc.tile_pool(name="persist", bufs=1) as pers,
            tc.tile_pool(name="xpool", bufs=4) as xp,
            tc.tile_pool(name="hpool", bufs=4) as hp,
            tc.tile_pool(name="ypool", bufs=3) as yp,
            tc.tile_pool(name="ph", bufs=4, space="PSUM") as php,
            tc.tile_pool(name="py", bufs=4, space="PSUM") as pyp,
        ):
            w1s = pers.tile([_P, _W1COLS], dt.bfloat16, name="w1s")
            w2s = pers.tile([_P, _W2COLS], dt.bfloat16, name="w2s")
            bs = pers.tile([_P, _E * _FT], dt.float32, name="bs")

            # HAM warm-up: the PE clock sits at 1.2 GHz until ~3.4 us
            # of sustained activity. Fill the otherwise-idle DMA ramp
            # (~8.5 us before the first real matmul) with dummy matmuls
            # on a memset tile so real work starts at 2.4 GHz.
            warm = pers.tile([_P, _P], dt.bfloat16, name="warm")
            nc.vector.memset(warm[:, :], 0.0)
            wp = php.tile([_P, _NB], dt.float32, name="ph", tag="ph")
            for _ in range(32):
                nc.tensor.matmul(wp[:, :_P], lhsT=warm[:, :],
                                 rhs=warm[:, :], start=True, stop=True)

            xs_tiles = [xp.tile([_P, _KD * _NB], dt.bfloat16,
                                name="xs", tag="xs") for _ in blocks]

            def dma_xs(bi):
                e, c0, nb = blocks[bi]
                nc.sync.dma_start(
                    out=xs_tiles[bi][:, :_KD * nb].rearrange(
                        "p (k c) -> p k c", k=_KD),
                    in_=xTv[:, :, c0:c0 + nb])

            def dma_w1(e, nchunks):
                off, span = e * _FT * _KD * _P, _FT * _KD * _P
                cw = span // nchunks
                for i in range(nchunks):
                    nc.sync.dma_start(
                        out=w1s[:, off + i * cw: off + (i + 1) * cw],
                        in_=w1[:, off + i * cw: off + (i + 1) * cw])

            def dma_w2(e, nchunks):
                off, span = e * _KD * _FT * _P, _KD * _FT * _P
                cw = span // nchunks
                for i in range(nchunks):
                    nc.sync.dma_start(
                        out=w2s[:, off + i * cw: off + (i + 1) * cw],
                        in_=w2[:, off + i * cw: off + (i + 1) * cw])

            # DMA triggers execute SERIALLY on the sync engine in
            # program order, and a trigger blocks the stream until its
            # wait clears. So: never front-load the whole input stream
            # — keep only a _LOOKAHEAD-block input window ahead of
            # compute, and issue each block's output DMA right after
            # its compute so it triggers as soon as its data is ready.
            issued = set()

            def issue_inputs(bi):
                e = blocks[bi][0]
                if e not in issued:
                    dma_w1(e, nchunks=1)
                    dma_w2(e, nchunks=1)
                    issued.add(e)
                dma_xs(bi)

            _LOOKAHEAD = 4
            e0 = blocks[0][0]
            issued.add(e0)
            dma_xs(0)
            # f-chunked w1 with the second x block right after chunk 0:
            # x is work-dense (13 us of PE work per MB), weights are not.
            off0, span0 = e0 * _FT * _KD * _P, _KD * _P
            nc.sync.dma_start(out=w1s[:, off0: off0 + span0],
                              in_=w1[:, off0: off0 + span0])
            if len(blocks) > 1:
                issue_inputs(1)
            for i in range(1, _FT):
                nc.sync.dma_start(
                    out=w1s[:, off0 + i * span0: off0 + (i + 1) * span0],
                    in_=w1[:, off0 + i * span0: off0 + (i + 1) * span0])
            nc.sync.dma_start(out=bs[:, :], in_=bia[:, :])
            dma_w2(e0, nchunks=2)
            for bi in range(2, min(_LOOKAHEAD, len(blocks))):
                issue_inputs(bi)

            # Ramp burst: run matmul1 of the first two blocks before the
            # first matmul2 — converts early DMA bytes into PE work
            # while w2[e0] is still in flight.
            _BURST = min(2, len(blocks))
            sched = [(b, "m1") for b in range(_BURST)]
            sched += [(b, "m2") for b in range(_BURST)]
            for b in range(_BURST, len(blocks)):
                sched += [(b, "m1"), (b, "m2")]

            hts_tiles = {}
            for bi, phase in sched:
                e, c0, nb = blocks[bi]
                if phase == "m1":
                    if bi + _LOOKAHEAD < len(blocks):
                        issue_inputs(bi + _LOOKAHEAD)
                    xs = xs_tiles[bi]
                    hts = hp.tile([_P, _FT * _NB], dt.bfloat16,
                                  name="hts", tag="hts")
                    hts_tiles[bi] = hts
                    for f in range(_FT):
                        ph = php.tile([_P, _NB], dt.float32,
                                      name="ph", tag="ph")
                        woff = (e * _FT + f) * _KD * _P
                        for k in range(_KD):
                            nc.tensor.matmul(
                                ph[:, :nb],
                                lhsT=w1s[:, woff + k * _P:
                                         woff + (k + 1) * _P],
                                rhs=xs[:, k * nb:(k + 1) * nb],
                                start=(k == 0), stop=(k == _KD - 1))
                        nc.scalar.activation(
                            hts[:, f * _NB: f * _NB + nb], ph[:, :nb],
                            AF.Gelu, bias=bs[:, e * _FT + f: e * _FT + f + 1])
                    continue
                hts = hts_tiles.pop(bi)
                last = bi == len(blocks) - 1
                yt = yp.tile([_P, _KD * _NB], dt.bfloat16, name="yt", tag="yt")
                for d in range(_KD):
                    # The very last output gates the kernel tail; split
                    # it column-wise so half drains earlier.
                    if last and d == _KD - 1 and nb >= 64:
                        h1 = (nb // 2 + 7) // 8 * 8
                        spans = [(0, h1), (h1, nb - h1)]
                    else:
                        spans = [(0, nb)]
                    for (s0, cw) in spans:
                        py = pyp.tile([_P, _NB], dt.float32,
                                      name="py", tag="py")
                        voff = (e * _KD + d) * _FT * _P
                        for ft in range(_FT):
                            nc.tensor.matmul(
                                py[:, :cw],
                                lhsT=w2s[:, voff + ft * _P:
                                         voff + (ft + 1) * _P],
                                rhs=hts[:, ft * _NB + s0: ft * _NB + s0 + cw],
                                start=(ft == 0), stop=(ft == _FT - 1))
                        nc.vector.tensor_scalar_add(
                            yt[:, d * nb + s0: d * nb + s0 + cw],
                            py[:, :cw], 0.0)
                        if last:
                            nc.sync.dma_start(
                                out=yT[d * _P:(d + 1) * _P,
                                       c0 + s0:c0 + s0 + cw],
                                in_=yt[:, d * nb + s0: d * nb + s0 + cw])
                if not last:
                    nc.sync.dma_start(
                        out=yTv[:, :, c0:c0 + nb],
                        in_=yt[:, :_KD * nb].rearrange(
                            "p (d c) -> p d c", d=_KD))
    nc.finalize()
    return nc


def kernel(x, expert_indices, w1, b1, w2, b2):
    global LAST_RESULT
    _ensure_path()
    _ensure_axon_hooks()
    import ml_dtypes
    from concourse.bass_utils import run_bass_kernel_spmd

    bf16 = ml_dtypes.bfloat16
    x = np.asarray(x)
    idxs = np.asarray(expert_indices)
    w1 = np.asarray(w1, dtype=np.float32)
    b1 = np.asarray(b1, dtype=np.float32)
    w2 = np.asarray(w2, dtype=np.float32)
    b2 = np.asarray(b2, dtype=np.float32)

    B, S, D = x.shape
    T = B * S
    E = w1.shape[0]
    K = idxs.shape[-1]
    assert D == _D and w1.shape[2] == _F and E == _E

    xf = np.ascontiguousarray(x.reshape(T, D).astype(np.float32))
    idx = idxs.reshape(T, K)

    # Deduplicated routing: one row per (token, expert) with integer
    # multiplicity (a token picking the same expert in several top-k
    # slots is computed once and scaled at the scatter).
    toks, wts, offs, cnts = [], [], [], []
    off = 0
    for e in range(E):
        m = (idx == e).sum(axis=1)
        te = np.nonzero(m)[0]
        toks.append(te)
        wts.append(m[te].astype(np.float32))
        cnts.append(len(te))
        offs.append(off)
        off += (len(te) + 7) // 8 * 8
    segs = tuple((c + 7) // 8 * 8 for c in cnts)
    Ctot = sum(segs)

    xTfull = np.zeros((_D, Ctot), dtype=bf16)
    for e in range(E):
        if cnts[e]:
            xTfull[:, offs[e]:offs[e] + cnts[e]] = \
                xf[toks[e]].T.astype(bf16)

    # Per-core weight shards: core c takes F columns [c*512, (c+1)*512)
    # of every expert, pre-arranged into the exact SBUF layouts:
    #   w1s: [p, e, f(4), k(8), 128]   (lhsT tiles for matmul1)
    #   w2s: [p, e, d(8), ft(4), 128]  (lhsT tiles for matmul2)
    #   bs : [p, e, f(4)]              (b1 per-partition scalars)
    in_maps = []
    for c in range(_NCORES):
        fs = slice(c * _FS, (c + 1) * _FS)
        a = w1[:, :, fs].reshape(E, _KD, _P, _FT, _P)
        w1shard = np.ascontiguousarray(
            a.transpose(2, 0, 3, 1, 4).reshape(_P, _W1COLS)).astype(bf16)
        b = w2[:, fs, :].reshape(E, _FT, _P, _KD, _P)
        w2shard = np.ascontiguousarray(
            b.transpose(2, 0, 3, 1, 4).reshape(_P, _W2COLS)).astype(bf16)
        bshard = np.ascontiguousarray(
            b1[:, fs].reshape(E, _FT, _P).transpose(2, 0, 1)
            .reshape(_P, E * _FT)).astype(np.float32)
        in_maps.append({"xT": xTfull, "w1": w1shard, "w2": w2shard,
                        "bias": bshard})

    nc = _cache.get(segs)
    if nc is None:
        nc = _build(segs)
        _cache[segs] = nc

    res = run_bass_kernel_spmd(nc, in_maps, core_ids=list(range(_NCORES)))
    LAST_RESULT = res

    ysum = np.zeros((_D, Ctot), dtype=np.float32)
    for c in range(_NCORES):
        ysum += np.asarray(res.results[c]["yT"]).astype(np.float32)

    out = np.zeros((T, D), dtype=np.float32)
    for e in range(E):
        n = cnts[e]
        if n:
            out[toks[e]] += wts[e][:, None] * (
                ysum[:, offs[e]:offs[e] + n].T + b2[e][None, :])
    return out.reshape(B, S, D)


# revision 11
# speedup vs baseline: 1.0427x; 1.0100x over previous
"""MoE expert-pool kernel for 8 TRN2 NeuronCores (hidden-dim sharded).

Strategy (v2, F-sharded):
  - Instead of one expert per core (which pads every core to the max
    expert token count), each core holds an F/8 = 512-wide slice of
    ALL 8 experts' weights and processes ALL routed (token, expert)
    rows, producing a partial y contracted over its F-slice. The host
    sums the 8 bf16 partials. Per-core PE work is exactly
    total_rows/8 column-equivalents regardless of routing skew.
  - Host-side routing dedups (token, expert) pairs: a token that picks
    the same expert in multiple top-k slots becomes ONE row with an
    integer multiplicity applied at the host scatter. Rows are sorted
    by expert into contiguous segments; segment sizes are baked into
    the NEFF (compiled per routing instance, cached by segment tuple).
  - On-device layout is transposed (feature axis on partitions, tokens
    on the free axis): hT = gelu(w1s^T x + b1s), yT_partial = w2s^T hT.
    b2 is added on the host (one add per row during the scatter).
  - x is streamed per 512-column block through a rotating SBUF pool
    (the full routed xT no longer fits alongside both weight shards);
    weight slices are DMA'd per expert just ahead of their segment.
"""

import numpy as np

_REPO = "/opt/trn_rl_repo"

_D = 1024  # d_model
_F = 4096  # d_ff
_P = 128   # partitions
_E = 8     # experts
_KD = _D // _P        # 8 contraction tiles along D
_FS = _F // 8         # 512: per-core F-slice width
_FT = _FS // _P       # 4 f-tiles per core
_NB = 512             # token block = one fp32 PSUM bank
_W1COLS = _E * _FT * _KD * _P   # w1 shard SBUF cols  [e][f][k][128]
_W2COLS = _E * _KD * _FT * _P   # w2 shard SBUF cols  [e][d][ft][128]

_NCORES = 8

_cache = {}
LAST_RESULT = None


def _ensure_path():
    import sys
    if _REPO not in sys.path:
        sys.path.insert(0, _REPO)


def _ensure_axon_hooks():
    """The container's `antenv` stub lacks `axon_hooks`, which
    bass_utils imports unconditionally on the traced (BASS_TRACE) axon
    path. Provide the missing get/set registry and register the NTFF
    ctypes hook the boot shim would have installed."""
    try:
        import antenv.axon_hooks  # noqa: F401
        return
    except ImportError:
        pass
    import sys
    import types
    mod = types.ModuleType("antenv.axon_hooks")
    mod._hook = None

    def set_axon_ntff_profile_hook(h):
        mod._hook = h

    def get_axon_ntff_profile_hook():
        return mod._hook

    mod.set_axon_ntff_profile_hook = set_axon_ntff_profile_hook
    mod.get_axon_ntff_profile_hook = get_axon_ntff_profile_hook
    sys.modules["antenv.axon_hooks"] = mod
    try:
        import antenv
        antenv.axon_hooks = mod
    except ImportError:
        pass
    try:
        from trn_agent_boot.trn_boot import _ntff_profile_via_ctypes
        hook = _ntff_profile_via_ctypes("/opt/axon/libaxon_pjrt.so")
        if hook is not None:
            mod._hook = hook
    except Exception:
        pass


_FIRSTNB = 160  # small leading block: PE starts after ~0.6 MB of DMA


def _plan_blocks(segs):
    """Split each expert segment into <=512-col blocks (one fp32 PSUM
    bank), sizes equal-ish multiples of 8. The very first block is
    carved small so matmul1 starts early in the DMA ramp.
    Returns [(e, col0, nb)]."""
    blocks = []
    col = 0
    first = True
    for e, se in enumerate(segs):
        if se == 0:
            continue
        sizes = []
        nblk = -(-se // _NB)
        base = se // nblk // 8 * 8
        sizes += [base] * nblk
        extra = se - base * nblk
        i = 0
        while extra > 0:
            step = min(8, extra)
            sizes[len(sizes) - nblk + i % nblk] += step
            extra -= step
            i += 1
        for nb in sizes:
            blocks.append((e, col, nb))
            col += nb
    return blocks, col


def _build(segs):
    _ensure_path()
    from concourse import bacc, mybir
    from concourse.tile import TileContext

    dt = mybir.dt
    AF = mybir.ActivationFunctionType

    blocks, Ctot = _plan_blocks(segs)
    assert blocks, "no routed tokens"

    # Bacc (not plain Bass): its compile() pass splits multi-sem waits
    # into event-semaphore instructions (TRN2 allows 1 wait/instruction).
    nc = bacc.Bacc("TRN2", target_bir_lowering=False, debug=False)
    xT = nc.declare_dram_parameter("xT", [_D, Ctot], dt.bfloat16, isOutput=False)
    w1 = nc.declare_dram_parameter("w1", [_P, _W1COLS], dt.bfloat16,
                                   isOutput=False)
    w2 = nc.declare_dram_parameter("w2", [_P, _W2COLS], dt.bfloat16,
                                   isOutput=False)
    bia = nc.declare_dram_parameter("bias", [_P, _E * _FT], dt.float32,
                                    isOutput=False)
    yT = nc.declare_dram_parameter("yT", [_D, Ctot], dt.bfloat16, isOutput=True)

    xTv = xT.rearrange("(k p) c -> p k c", p=_P)
    yTv = yT.rearrange("(d p) c -> p d c", p=_P)

    with TileContext(nc) as tc:
        with (
            tc.tile_pool(name="persist", bufs=1) as pers,
            tc.tile_pool(name="xpool", bufs=5) as xp,
            tc.tile_pool(name="hpool", bufs=2) as hp,
            tc.tile_pool(name="ypool", bufs=3) as yp,
            tc.tile_pool(name="ph", bufs=4, space="PSUM") as php,
            tc.tile_pool(name="py", bufs=4, space="PSUM") as pyp,
        ):
            w1s = pers.tile([_P, _W1COLS], dt.bfloat16, name="w1s")
            w2s = pers.tile([_P, _W2COLS], dt.bfloat16, name="w2s")
            bs = pers.tile([_P, _E * _FT], dt.float32, name="bs")

            # HAM warm-up: the PE clock sits at 1.2 GHz until ~3.4 us
            # of sustained activity, and the first real matmul cannot
            # start before ~13.5 us (supply-bound). Fill the idle ramp
            # with dummy matmuls so real work starts at 2.4 GHz; sized
            # to end just before the supply lands.
            warm = pers.tile([_P, _P], dt.bfloat16, name="warm")
            nc.vector.memset(warm[:, :], 0.0)
            wp = php.tile([_P, _NB], dt.float32, name="ph", tag="ph")
            for _ in range(76):
                nc.tensor.matmul(wp[:, :_P], lhsT=warm[:, :],
                                 rhs=warm[:, :], start=True, stop=True)

            xs_tiles = [xp.tile([_P, _KD * _NB], dt.bfloat16,
                                name="xs", tag="xs") for _ in blocks]

            def dma_xs(bi):
                e, c0, nb = blocks[bi]
                nc.sync.dma_start(
                    out=xs_tiles[bi][:, :_KD * nb].rearrange(
                        "p (k c) -> p k c", k=_KD),
                    in_=xTv[:, :, c0:c0 + nb])

            def dma_w1(e, nchunks):
                off, span = e * _FT * _KD * _P, _FT * _KD * _P
                cw = span // nchunks
                for i in range(nchunks):
                    nc.sync.dma_start(
                        out=w1s[:, off + i * cw: off + (i + 1) * cw],
                        in_=w1[:, off + i * cw: off + (i + 1) * cw])

            def dma_w2(e, nchunks):
                off, span = e * _KD * _FT * _P, _KD * _FT * _P
                cw = span // nchunks
                for i in range(nchunks):
                    nc.sync.dma_start(
                        out=w2s[:, off + i * cw: off + (i + 1) * cw],
                        in_=w2[:, off + i * cw: off + (i + 1) * cw])

            # DMA triggers execute SERIALLY on the sync engine in
            # program order, and a trigger blocks the stream until its
            # wait clears. So: never front-load the whole input stream
            # — keep only a _LOOKAHEAD-block input window ahead of
            # compute, and issue each block's output DMA right after
            # its compute so it triggers as soon as its data is ready.
            issued = set()

            def issue_inputs(bi):
                e = blocks[bi][0]
                if e not in issued:
                    dma_w1(e, nchunks=1)
                    dma_w2(e, nchunks=1)
                    issued.add(e)
                dma_xs(bi)

            _LOOKAHEAD = 5
            e0 = blocks[0][0]
            dma_xs(0)
            dma_w1(e0, nchunks=_FT)
            nc.sync.dma_start(out=bs[:, :], in_=bia[:, :])
            dma_w2(e0, nchunks=2)
            issued.add(e0)
            for bi in range(1, min(_LOOKAHEAD, len(blocks))):
                issue_inputs(bi)

            sched = [(b, ph) for b in range(len(blocks))
                     for ph in ("m1", "m2")]

            hts_tiles = {}
            for bi, phase in sched:
                e, c0, nb = blocks[bi]
                if phase == "m1":
                    if bi + _LOOKAHEAD < len(blocks):
                        issue_inputs(bi + _LOOKAHEAD)
                    xs = xs_tiles[bi]
                    hts = hp.tile([_P, _FT * _NB], dt.bfloat16,
                                  name="hts", tag="hts")
                    hts_tiles[bi] = hts
                    for f in range(_FT):
                        ph = php.tile([_P, _NB], dt.float32,
                                      name="ph", tag="ph")
                        woff = (e * _FT + f) * _KD * _P
                        for k in range(_KD):
                            nc.tensor.matmul(
                                ph[:, :nb],
                                lhsT=w1s[:, woff + k * _P:
                                         woff + (k + 1) * _P],
                                rhs=xs[:, k * nb:(k + 1) * nb],
                                start=(k == 0), stop=(k == _KD - 1))
                        nc.scalar.activation(
                            hts[:, f * _NB: f * _NB + nb], ph[:, :nb],
                            AF.Gelu, bias=bs[:, e * _FT + f: e * _FT + f + 1])
                    continue
                hts = hts_tiles.pop(bi)
                last = bi == len(blocks) - 1
                yt = yp.tile([_P, _KD * _NB], dt.bfloat16, name="yt", tag="yt")
                for d in range(_KD):
                    # The very last output gates the kernel tail; split
                    # it column-wise so half drains earlier.
                    if last and d == _KD - 1 and nb >= 64:
                        h1 = (nb // 2 + 7) // 8 * 8
                        spans = [(0, h1), (h1, nb - h1)]
                    else:
                        spans = [(0, nb)]
                    for (s0, cw) in spans:
                        py = pyp.tile([_P, _NB], dt.float32,
                                      name="py", tag="py")
                        voff = (e * _KD + d) * _FT * _P
                        for ft in range(_FT):
                            nc.tensor.matmul(
                                py[:, :cw],
                                lhsT=w2s[:, voff + ft * _P:
                                         voff + (ft + 1) * _P],
                                rhs=hts[:, ft * _NB + s0: ft * _NB + s0 + cw],
                                start=(ft == 0), stop=(ft == _FT - 1))
                        nc.vector.tensor_scalar_add(
                            yt[:, d * nb + s0: d * nb + s0 + cw],
                            py[:, :cw], 0.0)
                        if last:
                            nc.sync.dma_start(
                                out=yT[d * _P:(d + 1) * _P,
                                       c0 + s0:c0 + s0 + cw],
                                in_=yt[:, d * nb + s0: d * nb + s0 + cw])
                if not last:
                    nc.sync.dma_start(
                        out=yTv[:, :, c0:c0 + nb],
                        in_=yt[:, :_KD * nb].rearrange(
                            "p (d c) -> p d c", d=_KD))
    nc.finalize()
    return nc


def kernel(x, expert_indices, w1, b1, w2, b2):
    global LAST_RESULT
    _ensure_path()
    _ensure_axon_hooks()
    import ml_dtypes
    from concourse.bass_utils import run_bass_kernel_spmd

    bf16 = ml_dtypes.bfloat16
    x = np.asarray(x)
    idxs = np.asarray(expert_indices)
    w1 = np.asarray(w1, dtype=np.float32)
    b1 = np.asarray(b1, dtype=np.float32)
    w2 = np.asarray(w2, dtype=np.float32)
    b2 = np.asarray(b2, dtype=np.float32)

    B, S, D = x.shape
    T = B * S
    E = w1.shape[0]
    K = idxs.shape[-1]
    assert D == _D and w1.shape[2] == _F and E == _E

    xf = np.ascontiguousarray(x.reshape(T, D).astype(np.float32))
    idx = idxs.reshape(T, K)

    # Deduplicated routing: one row per (token, expert) with integer
    # multiplicity (a token picking the same expert in several top-k
    # slots is computed once and scaled at the scatter).
    toks, wts, offs, cnts = [], [], [], []
    off = 0
    for e in range(E):
        m = (idx == e).sum(axis=1)
        te = np.nonzero(m)[0]
        toks.append(te)
        wts.append(m[te].astype(np.float32))
        cnts.append(len(te))
        offs.append(off)
        off += (len(te) + 7) // 8 * 8
    segs = tuple((c + 7) // 8 * 8 for c in cnts)
    Ctot = sum(segs)

    xTfull = np.zeros((_D, Ctot), dtype=bf16)
    for e in range(E):
        if cnts[e]:
            xTfull[:, offs[e]:offs[e] + cnts[e]] = \
                xf[toks[e]].T.astype(bf16)

    # Per-core weight shards: core c takes F columns [c*512, (c+1)*512)
    # of every expert, pre-arranged into the exact SBUF layouts:
    #   w1s: [p, e, f(4), k(8), 128]   (lhsT tiles for matmul1)
    #   w2s: [p, e, d(8), ft(4), 128]  (lhsT tiles for matmul2)
    #   bs : [p, e, f(4)]              (b1 per-partition scalars)
    in_maps = []
    for c in range(_NCORES):
        fs = slice(c * _FS, (c + 1) * _FS)
        a = w1[:, :, fs].reshape(E, _KD, _P, _FT, _P)
        w1shard = np.ascontiguousarray(
            a.transpose(2, 0, 3, 1, 4).reshape(_P, _W1COLS)).astype(bf16)
        b = w2[:, fs, :].reshape(E, _FT, _P, _KD, _P)
        w2shard = np.ascontiguousarray(
            b.transpose(2, 0, 3, 1, 4).reshape(_P, _W2COLS)).astype(bf16)
        bshard = np.ascontiguousarray(
            b1[:, fs].reshape(E, _FT, _P).transpose(2, 0, 1)
            .reshape(_P, E * _FT)).astype(np.float32)
        in_maps.append({"xT": xTfull, "w1": w1shard, "w2": w2shard,
                        "bias": bshard})

    nc = _cache.get(segs)
    if nc is None:
        nc = _build(segs)
        _cache[segs] = nc

    res = run_bass_kernel_spmd(nc, in_maps, core_ids=list(range(_NCORES)))
    LAST_RESULT = res

    ysum = np.zeros((_D, Ctot), dtype=np.float32)
    for c in range(_NCORES):
        ysum += np.asarray(res.results[c]["yT"]).astype(np.float32)

    out = np.zeros((T, D), dtype=np.float32)
    for e in range(E):
        n = cnts[e]
        if n:
            out[toks[e]] += wts[e][:, None] * (
                ysum[:, offs[e]:offs[e] + n].T + b2[e][None, :])
    return out.reshape(B, S, D)


# revision 12
# speedup vs baseline: 1.0464x; 1.0036x over previous
"""MoE expert-pool kernel for 8 TRN2 NeuronCores (hidden-dim sharded).

Strategy (v2, F-sharded):
  - Instead of one expert per core (which pads every core to the max
    expert token count), each core holds an F/8 = 512-wide slice of
    ALL 8 experts' weights and processes ALL routed (token, expert)
    rows, producing a partial y contracted over its F-slice. The host
    sums the 8 bf16 partials. Per-core PE work is exactly
    total_rows/8 column-equivalents regardless of routing skew.
  - Host-side routing dedups (token, expert) pairs: a token that picks
    the same expert in multiple top-k slots becomes ONE row with an
    integer multiplicity applied at the host scatter. Rows are sorted
    by expert into contiguous segments; segment sizes are baked into
    the NEFF (compiled per routing instance, cached by segment tuple).
  - On-device layout is transposed (feature axis on partitions, tokens
    on the free axis): hT = gelu(w1s^T x + b1s), yT_partial = w2s^T hT.
    b2 is added on the host (one add per row during the scatter).
  - x is streamed per 512-column block through a rotating SBUF pool
    (the full routed xT no longer fits alongside both weight shards);
    weight slices are DMA'd per expert just ahead of their segment.
"""

import numpy as np

_REPO = "/opt/trn_rl_repo"

_D = 1024  # d_model
_F = 4096  # d_ff
_P = 128   # partitions
_E = 8     # experts
_KD = _D // _P        # 8 contraction tiles along D
_FS = _F // 8         # 512: per-core F-slice width
_FT = _FS // _P       # 4 f-tiles per core
_NB = 512             # token block = one fp32 PSUM bank
_W1COLS = _E * _FT * _KD * _P   # w1 shard SBUF cols  [e][f][k][128]
_W2COLS = _E * _KD * _FT * _P   # w2 shard SBUF cols  [e][d][ft][128]

_NCORES = 8

_cache = {}
LAST_RESULT = None


def _ensure_path():
    import sys
    if _REPO not in sys.path:
        sys.path.insert(0, _REPO)


def _ensure_axon_hooks():
    """The container's `antenv` stub lacks `axon_hooks`, which
    bass_utils imports unconditionally on the traced (BASS_TRACE) axon
    path. Provide the missing get/set registry and register the NTFF
    ctypes hook the boot shim would have installed."""
    try:
        import antenv.axon_hooks  # noqa: F401
        return
    except ImportError:
        pass
    import sys
    import types
    mod = types.ModuleType("antenv.axon_hooks")
    mod._hook = None

    def set_axon_ntff_profile_hook(h):
        mod._hook = h

    def get_axon_ntff_profile_hook():
        return mod._hook

    mod.set_axon_ntff_profile_hook = set_axon_ntff_profile_hook
    mod.get_axon_ntff_profile_hook = get_axon_ntff_profile_hook
    sys.modules["antenv.axon_hooks"] = mod
    try:
        import antenv
        antenv.axon_hooks = mod
    except ImportError:
        pass
    try:
        from trn_agent_boot.trn_boot import _ntff_profile_via_ctypes
        hook = _ntff_profile_via_ctypes("/opt/axon/libaxon_pjrt.so")
        if hook is not None:
            mod._hook = hook
    except Exception:
        pass


_FIRSTNB = 160  # small leading block: PE starts after ~0.6 MB of DMA


def _plan_blocks(segs):
    """Split each expert segment into <=512-col blocks (one fp32 PSUM
    bank), sizes equal-ish multiples of 8. The very first block is
    carved small so matmul1 starts early in the DMA ramp.
    Returns [(e, col0, nb)]."""
    blocks = []
    col = 0
    first = True
    for e, se in enumerate(segs):
        if se == 0:
            continue
        sizes = []
        nblk = -(-se // _NB)
        base = se // nblk // 4 * 4
        sizes += [base] * nblk
        extra = se - base * nblk
        i = 0
        while extra > 0:
            step = min(4, extra)
            sizes[len(sizes) - nblk + i % nblk] += step
            extra -= step
            i += 1
        for nb in sizes:
            blocks.append((e, col, nb))
            col += nb
    return blocks, col


def _build(segs):
    _ensure_path()
    from concourse import bacc, mybir
    from concourse.tile import TileContext

    dt = mybir.dt
    AF = mybir.ActivationFunctionType

    blocks, Ctot = _plan_blocks(segs)
    assert blocks, "no routed tokens"

    # Bacc (not plain Bass): its compile() pass splits multi-sem waits
    # into event-semaphore instructions (TRN2 allows 1 wait/instruction).
    nc = bacc.Bacc("TRN2", target_bir_lowering=False, debug=False)
    xT = nc.declare_dram_parameter("xT", [_D, Ctot], dt.bfloat16, isOutput=False)
    w1 = nc.declare_dram_parameter("w1", [_P, _W1COLS], dt.bfloat16,
                                   isOutput=False)
    w2 = nc.declare_dram_parameter("w2", [_P, _W2COLS], dt.bfloat16,
                                   isOutput=False)
    bia = nc.declare_dram_parameter("bias", [_P, _E * _FT], dt.float32,
                                    isOutput=False)
    yT = nc.declare_dram_parameter("yT", [_D, Ctot], dt.bfloat16, isOutput=True)

    xTv = xT.rearrange("(k p) c -> p k c", p=_P)
    yTv = yT.rearrange("(d p) c -> p d c", p=_P)

    with TileContext(nc) as tc:
        with (
            tc.tile_pool(name="persist", bufs=1) as pers,
            tc.tile_pool(name="xpool", bufs=5) as xp,
            tc.tile_pool(name="hpool", bufs=2) as hp,
            tc.tile_pool(name="ypool", bufs=3) as yp,
            tc.tile_pool(name="ph", bufs=4, space="PSUM") as php,
            tc.tile_pool(name="py", bufs=4, space="PSUM") as pyp,
        ):
            w1s = pers.tile([_P, _W1COLS], dt.bfloat16, name="w1s")
            w2s = pers.tile([_P, _W2COLS], dt.bfloat16, name="w2s")
            bs = pers.tile([_P, _E * _FT], dt.float32, name="bs")

            # HAM warm-up: the PE clock sits at 1.2 GHz until ~3.4 us
            # of sustained activity, and the first real matmul cannot
            # start before ~13.5 us (supply-bound). Fill the idle ramp
            # with dummy matmuls so real work starts at 2.4 GHz; sized
            # to end just before the supply lands.
            warm = pers.tile([_P, _P], dt.bfloat16, name="warm")
            nc.vector.memset(warm[:, :], 0.0)
            wp = php.tile([_P, _NB], dt.float32, name="ph", tag="ph")
            for _ in range(64):
                nc.tensor.matmul(wp[:, :_P], lhsT=warm[:, :],
                                 rhs=warm[:, :], start=True, stop=True)

            xs_tiles = [xp.tile([_P, _KD * _NB], dt.bfloat16,
                                name="xs", tag="xs") for _ in blocks]

            def dma_xs(bi):
                e, c0, nb = blocks[bi]
                nc.sync.dma_start(
                    out=xs_tiles[bi][:, :_KD * nb].rearrange(
                        "p (k c) -> p k c", k=_KD),
                    in_=xTv[:, :, c0:c0 + nb])

            def dma_w1(e, nchunks):
                off, span = e * _FT * _KD * _P, _FT * _KD * _P
                cw = span // nchunks
                for i in range(nchunks):
                    nc.sync.dma_start(
                        out=w1s[:, off + i * cw: off + (i + 1) * cw],
                        in_=w1[:, off + i * cw: off + (i + 1) * cw])

            def dma_w2(e, nchunks):
                off, span = e * _KD * _FT * _P, _KD * _FT * _P
                cw = span // nchunks
                for i in range(nchunks):
                    nc.sync.dma_start(
                        out=w2s[:, off + i * cw: off + (i + 1) * cw],
                        in_=w2[:, off + i * cw: off + (i + 1) * cw])

            # DMA triggers execute SERIALLY on the sync engine in
            # program order, and a trigger blocks the stream until its
            # wait clears. So: never front-load the whole input stream
            # — keep only a _LOOKAHEAD-block input window ahead of
            # compute, and issue each block's output DMA right after
            # its compute so it triggers as soon as its data is ready.
            issued = set()

            def issue_inputs(bi):
                e = blocks[bi][0]
                if e not in issued:
                    dma_w1(e, nchunks=1)
                    dma_w2(e, nchunks=1)
                    issued.add(e)
                dma_xs(bi)

            _LOOKAHEAD = 5
            e0 = blocks[0][0]
            dma_xs(0)
            dma_w1(e0, nchunks=_FT)
            nc.sync.dma_start(out=bs[:, :], in_=bia[:, :])
            dma_w2(e0, nchunks=2)
            issued.add(e0)
            for bi in range(1, min(_LOOKAHEAD, len(blocks))):
                issue_inputs(bi)

            sched = [(b, ph) for b in range(len(blocks))
                     for ph in ("m1", "m2")]

            hts_tiles = {}
            for bi, phase in sched:
                e, c0, nb = blocks[bi]
                if phase == "m1":
                    if bi + _LOOKAHEAD < len(blocks):
                        issue_inputs(bi + _LOOKAHEAD)
                    xs = xs_tiles[bi]
                    hts = hp.tile([_P, _FT * _NB], dt.bfloat16,
                                  name="hts", tag="hts")
                    hts_tiles[bi] = hts
                    for f in range(_FT):
                        ph = php.tile([_P, _NB], dt.float32,
                                      name="ph", tag="ph")
                        woff = (e * _FT + f) * _KD * _P
                        for k in range(_KD):
                            nc.tensor.matmul(
                                ph[:, :nb],
                                lhsT=w1s[:, woff + k * _P:
                                         woff + (k + 1) * _P],
                                rhs=xs[:, k * nb:(k + 1) * nb],
                                start=(k == 0), stop=(k == _KD - 1))
                        nc.scalar.activation(
                            hts[:, f * _NB: f * _NB + nb], ph[:, :nb],
                            AF.Gelu, bias=bs[:, e * _FT + f: e * _FT + f + 1])
                    continue
                hts = hts_tiles.pop(bi)
                last = bi == len(blocks) - 1
                yt = yp.tile([_P, _KD * _NB], dt.bfloat16, name="yt", tag="yt")
                for d in range(_KD):
                    # The very last output gates the kernel tail; split
                    # it column-wise so half drains earlier.
                    if last and d == _KD - 1 and nb >= 128:
                        spans = [(0, nb - 64), (nb - 64, 64)]
                    else:
                        spans = [(0, nb)]
                    for (s0, cw) in spans:
                        py = pyp.tile([_P, _NB], dt.float32,
                                      name="py", tag="py")
                        voff = (e * _KD + d) * _FT * _P
                        for ft in range(_FT):
                            nc.tensor.matmul(
                                py[:, :cw],
                                lhsT=w2s[:, voff + ft * _P:
                                         voff + (ft + 1) * _P],
                                rhs=hts[:, ft * _NB + s0: ft * _NB + s0 + cw],
                                start=(ft == 0), stop=(ft == _FT - 1))
                        nc.vector.tensor_scalar_add(
                            yt[:, d * nb + s0: d * nb + s0 + cw],
                            py[:, :cw], 0.0)
                        if last:
                            nc.sync.dma_start(
                                out=yT[d * _P:(d + 1) * _P,
                                       c0 + s0:c0 + s0 + cw],
                                in_=yt[:, d * nb + s0: d * nb + s0 + cw])
                if not last:
                    nc.sync.dma_start(
                        out=yTv[:, :, c0:c0 + nb],
                        in_=yt[:, :_KD * nb].rearrange(
                            "p (d c) -> p d c", d=_KD))
    nc.finalize()
    return nc


def kernel(x, expert_indices, w1, b1, w2, b2):
    global LAST_RESULT
    _ensure_path()
    _ensure_axon_hooks()
    import ml_dtypes
    from concourse.bass_utils import run_bass_kernel_spmd

    bf16 = ml_dtypes.bfloat16
    x = np.asarray(x)
    idxs = np.asarray(expert_indices)
    w1 = np.asarray(w1, dtype=np.float32)
    b1 = np.asarray(b1, dtype=np.float32)
    w2 = np.asarray(w2, dtype=np.float32)
    b2 = np.asarray(b2, dtype=np.float32)

    B, S, D = x.shape
    T = B * S
    E = w1.shape[0]
    K = idxs.shape[-1]
    assert D == _D and w1.shape[2] == _F and E == _E

    xf = np.ascontiguousarray(x.reshape(T, D).astype(np.float32))
    idx = idxs.reshape(T, K)

    # Deduplicated routing: one row per (token, expert) with integer
    # multiplicity (a token picking the same expert in several top-k
    # slots is computed once and scaled at the scatter).
    toks, wts, offs, cnts = [], [], [], []
    off = 0
    for e in range(E):
        m = (idx == e).sum(axis=1)
        te = np.nonzero(m)[0]
        toks.append(te)
        wts.append(m[te].astype(np.float32))
        cnts.append(len(te))
        offs.append(off)
        off += (len(te) + 3) // 4 * 4
    segs = tuple((c + 3) // 4 * 4 for c in cnts)
    Ctot = sum(segs)

    xTfull = np.zeros((_D, Ctot), dtype=bf16)
    for e in range(E):
        if cnts[e]:
            xTfull[:, offs[e]:offs[e] + cnts[e]] = \
                xf[toks[e]].T.astype(bf16)

    # Per-core weight shards: core c takes F columns [c*512, (c+1)*512)
    # of every expert, pre-arranged into the exact SBUF layouts:
    #   w1s: [p, e, f(4), k(8), 128]   (lhsT tiles for matmul1)
    #   w2s: [p, e, d(8), ft(4), 128]  (lhsT tiles for matmul2)
    #   bs : [p, e, f(4)]              (b1 per-partition scalars)
    in_maps = []
    for c in range(_NCORES):
        fs = slice(c * _FS, (c + 1) * _FS)
        a = w1[:, :, fs].reshape(E, _KD, _P, _FT, _P)
        w1shard = np.ascontiguousarray(
            a.transpose(2, 0, 3, 1, 4).reshape(_P, _W1COLS)).astype(bf16)
        b = w2[:, fs, :].reshape(E, _FT, _P, _KD, _P)
        w2shard = np.ascontiguousarray(
            b.transpose(2, 0, 3, 1, 4).reshape(_P, _W2COLS)).astype(bf16)
        bshard = np.ascontiguousarray(
            b1[:, fs].reshape(E, _FT, _P).transpose(2, 0, 1)
            .reshape(_P, E * _FT)).astype(np.float32)
        in_maps.append({"xT": xTfull, "w1": w1shard, "w2": w2shard,
                        "bias": bshard})

    nc = _cache.get(segs)
    if nc is None:
        nc = _build(segs)
        _cache[segs] = nc

    res = run_bass_kernel_spmd(nc, in_maps, core_ids=list(range(_NCORES)))
    LAST_RESULT = res

    ysum = np.zeros((_D, Ctot), dtype=np.float32)
    for c in range(_NCORES):
        ysum += np.asarray(res.results[c]["yT"]).astype(np.float32)

    out = np.zeros((T, D), dtype=np.float32)
    for e in range(E):
        n = cnts[e]
        if n:
            out[toks[e]] += wts[e][:, None] * (
                ysum[:, offs[e]:offs[e] + n].T + b2[e][None, :])
    return out.reshape(B, S, D)
